# revision 37
# baseline (speedup 1.0000x reference)
"""Single-head attention (B=4, S=2048, D=E=1024) on 8 trn2 NeuronCores.

Sharding: data-parallel over (batch, q-half) -> 8 shards. Each core gets a
1024-row q shard plus the full 2048 keys of its batch; K/V projections are
recomputed on both cores of a batch pair (zero collectives).

v5: PE-continuity design. One long gap-free matmul stream; DMA always ahead.
  - bf16 inputs (host-converted): halves DMA bytes + SBUF. Matmul rate is
    identical (1 cycle/row, keyed on moving-operand dtype).
  - single qp phase for BOTH q-halves (wq/qT loaded once)
  - ONE PSUM pool for the whole kernel (4 tags x ring-2 = 8 banks): per-slot
    ring deps instead of pool-close barriers (a close stalls the next
    phase's first matmul on the closed pool's LAST consumer)
  - DMA queues: wq alone on sync (SP owns the shared HWDGE's early window);
    qt/kT/wk/wv/ow on the gpsimd SWDGE in need order; smalls + half the out
    writes on scalar. The DMA pipe is serial and FIFO-by-gen-completion, so
    gen order == need order.
  - const-AP warm-up matmuls (zero deps) ramp the PE p-state during the
    ~4us DMA fill; count sized to end just before wq0 lands
  - softmax sums via fp8e4 DoubleRow (256-key contraction, 0.5 cyc/row):
    the 2048-term positive sum averages fp8 element error to ~0.1%. exp is
    written bf16 for ctx; DVE makes the fp8 shadow. Sum matmuls lag exp by
    two k-tiles; the final pair hides inside the ctx accumulation.
  - per q-block: logits -> exp -> ctx -> out fused; no DRAM ctx bounce;
    out staging in one persistent pool (close would barrier on DMA sems)

Per-core math (token-transposed on host so contraction lands on partitions):
  qp^T [E,q]   = (lhsT=wq[D,E], rhs=qT[D,q]) * (1/sqrt E) + bq/sqrt(E)
  kp^T [E,k]   = (lhsT=wk, rhs=kT) + bk
  vp   [k,E]   = (lhsT=vT[D,k], rhs=wv[D,E]) + bv
  lgT  [k,q]   = (lhsT=kp^T slice, rhs=qp^T)          (scale folded into qp)
  expT [k,q]   = Exp(lgT + mask*NEG)                  (ACT, per-partition bias)
  s    [.,q]   = ones-matmul over expT                (no max-sub: logits~N(0,1))
  ctx^T[E,q]   = (lhsT=vp slice, rhs=expT) * recip(s)
  out  [q,D]   = (lhsT=ctx^T slice, rhs=ow[E,D]) + ob
"""

import os
import numpy as np

P = 128
NEG = -1.0e9


def build_nc(D=1024, E=1024, SK=2048, QSH=1024, QB=512):
    """Build the per-core Bass module (SPMD; same program on all cores)."""
    import concourse.bass as bass
    import concourse.mybir as mybir
    import concourse.tile as tile
    from concourse import bacc

    f32 = mybir.dt.float32
    bf16 = mybir.dt.bfloat16
    AF = mybir.ActivationFunctionType

    DT = D // P          # contraction tiles over model dim
    ET = E // P          # enc tiles
    KT = SK // P         # key tiles
    NQB = QSH // QB      # q blocks
    KNB = 512            # key free-dim block for kp
    NKB = SK // KNB
    ENB = 512            # E free-dim block for vp
    DNB = 512            # model free-dim block for out
    MQ = QB // P
    ISCALE = 1.0 / float(np.sqrt(E))

    nc = bacc.Bacc(trn_type="TRN2")

    # ---- I/O ----
    qT = nc.dram_tensor("qT", [D, QSH], bf16, kind="ExternalInput")[:, :]
    kTd = nc.dram_tensor("kT", [D, SK], bf16, kind="ExternalInput")[:, :]
    vTd = nc.dram_tensor("vT", [D, SK], bf16, kind="ExternalInput")[:, :]
    mask_cols = nc.dram_tensor("mask_cols", [P, KT], f32, kind="ExternalInput")[:, :]
    ones_d = nc.dram_tensor("ones_d", [P, 256], bf16, kind="ExternalInput")[:, :]
    ones8_d = nc.dram_tensor("ones8_d", [P, 256], mybir.dt.float8e4,
                             kind="ExternalInput")[:, :]
    wq = nc.dram_tensor("wq", [D, E], bf16, kind="ExternalInput")[:, :]
    wk = nc.dram_tensor("wk", [D, E], bf16, kind="ExternalInput")[:, :]
    wv = nc.dram_tensor("wv", [D, E], bf16, kind="ExternalInput")[:, :]
    ow = nc.dram_tensor("ow", [E, D], bf16, kind="ExternalInput")[:, :]
    bq_col = nc.dram_tensor("bq_col", [P, ET], f32, kind="ExternalInput")[:, :]
    bk_col = nc.dram_tensor("bk_col", [P, ET], f32, kind="ExternalInput")[:, :]
    bv_bc = nc.dram_tensor("bv_bc", [P, E], f32, kind="ExternalInput")[:, :]
    ob_bc = nc.dram_tensor("ob_bc", [P, D], f32, kind="ExternalInput")[:, :]
    out = nc.dram_tensor("out", [QSH, D], f32, kind="ExternalOutput")[:, :]

    qT_r = qT.rearrange("(t p) n -> p t n", p=P)   # [128, DT, QSH]
    kT_r = kTd.rearrange("(t p) n -> p t n", p=P)
    vT_r = vTd.rearrange("(t p) n -> p t n", p=P)
    wq_r = wq.rearrange("(t p) n -> p t n", p=P)   # [128, DT, E]
    wk_r = wk.rearrange("(t p) n -> p t n", p=P)
    wv_r = wv.rearrange("(t p) n -> p t n", p=P)
    ow_r = ow.rearrange("(t p) n -> p t n", p=P)   # [128, ET, D]

    def mm(ps, lhsT, rhs, start, stop):
        nc.tensor.matmul(ps, lhsT, rhs, start=start, stop=stop)

    with tile.TileContext(nc) as tc:
        with tc.tile_pool(name="smalls", bufs=1) as smalls:
            # small persistent tensors on the scalar (ACT) DMA queue,
            # all emitted after the qt h0 halves (load_smalls_rest); mask/bq
            # arrive pre-multiplied by NEG / 1/sqrt(E) from the host.
            bq_t = smalls.tile([P, ET], f32, name="bqc")
            ones_t = smalls.tile([P, 256], bf16, name="ones")
            ones8_t = smalls.tile([P, 2, P], mybir.dt.float8e4, name="ones8")
            mask_t = smalls.tile([P, KT], f32, name="maskc")
            bk_t = smalls.tile([P, ET], f32, name="bkc")
            bv_t = smalls.tile([P, E], f32, name="bv_t")
            ob_t = smalls.tile([P, D], f32, name="ob_t")
            recip_ts = [smalls.tile([P, QB], f32, name=f"recip{i}")
                        for i in range(NQB)]

            def load_smalls_rest():
                # emitted after the qt h0 halves so they don't delay them
                nc.scalar.dma_start(bq_t[:], bq_col)
                nc.scalar.dma_start(ones_t[:], ones_d)
                nc.scalar.dma_start(ones8_t[:], ones8_d)
                nc.scalar.dma_start(mask_t[:], mask_cols)
                nc.scalar.dma_start(bk_t[:], bk_col)
                nc.scalar.dma_start(bv_t[:], bv_bc)
                nc.scalar.dma_start(ob_t[:], ob_bc)

            # warm-up matmuls read the preamble-materialized const-1.0 AP
            # (broadcast [128,1] -> [128,256]): zero runtime dependencies, so
            # the PE ramps its p-state from t~0 while the DMA pipe fills.
            wu_src = nc.const_aps.tensor(1.0, (P, 256), bf16)

            # ONE psum pool for the whole kernel: 4 tags x ring-2 = 8 banks.
            # Per-slot ring deps replace pool-close barriers (a pool close
            # stalls the next phase's first matmul on the closed pool's LAST
            # consumer; the ring only waits on the 2-back tile's consumers).
            ps8_cm = tc.tile_pool(name="ps8", bufs=2, space="PSUM")
            ps8 = ps8_cm.__enter__()
            for i in range(14):
                wt = ps8.tile([P, 256], f32, tag=f"t{i % 4}", name=f"wu{i}")
                mm(wt[:], wu_src[:, :P], wu_src, True, True)

            with tc.tile_pool(name="qppool", bufs=1) as qp_pool:
                qp = qp_pool.tile([P, ET, QSH], bf16, name="qp")
                with tc.tile_pool(name="kvp", bufs=1) as kvp:
                    kp = kvp.tile([P, ET, SK], bf16, name="kp")
                    vp = kvp.tile([P, KT, E], bf16, name="vp")

                    s1a_cm = tc.tile_pool(name="s1a", bufs=1)
                    s1b_cm = tc.tile_pool(name="s1b", bufs=2)
                    s1c_cm = tc.tile_pool(name="s1c", bufs=3)
                    s1a = s1a_cm.__enter__()
                    s1b = s1b_cm.__enter__()
                    s1c = s1c_cm.__enter__()
                    if True:
                        # gpsimd queue order (need order): qt h0, qt h1,
                        # kT b0, wk, kT b1, wv; kT b2/b3 are emitted inside
                        # the kp loop so their ring wait can't block wv/ow.
                        qt_t = []
                        for t in range(DT):
                            qt = s1a.tile([P, QSH], bf16, name=f"qt{t}")
                            nc.gpsimd.dma_start(qt[:, :QB], qT_r[:, t, :QB])
                            qt_t.append(qt)
                        for t in range(DT):
                            nc.gpsimd.dma_start(qt_t[t][:, QB:],
                                                qT_r[:, t, QB:])
                        load_smalls_rest()
                        kt_tiles = [s1b.tile([P, DT, KNB], bf16, tag="ktb",
                                             name=f"kt{nb}")
                                    for nb in range(NKB)]
                        # kT b0 before wk (single gen, needed at the same
                        # time); both after the qt halves so their transfers
                        # miss the qp phase's wq window
                        nc.gpsimd.dma_start(kt_tiles[0][:], kT_r[:, :, :KNB])
                        wk_t = []
                        for t in range(DT):
                            w = s1a.tile([P, E], bf16, name=f"wk{t}")
                            nc.gpsimd.dma_start(w[:], wk_r[:, t, :])
                            wk_t.append(w)
                        nc.gpsimd.dma_start(kt_tiles[1][:],
                                            kT_r[:, :, KNB:2 * KNB])
                        wv_t = []
                        for t in range(DT):
                            w = s1a.tile([P, E], bf16, name=f"wv{t}")
                            nc.gpsimd.dma_start(w[:], wv_r[:, t, :])
                            wv_t.append(w)
                        # vT ring tiles; DMAs go on the sync queue after wq
                        vt_tiles = []
                        for mp in range(KT // 2):
                            vt_ = s1c.tile([P, DT, 2 * P], bf16, tag="vt",
                                           name=f"vt{mp}")
                            vt_tiles.append(vt_)

                        # ---- qp phase: both q halves; wq on sync queue ----
                        with tc.tile_pool(name="s2", bufs=1) as s2:
                            wq_t = []
                            for t in range(DT):
                                w = s2.tile([P, E], bf16, name=f"wq{t}")
                                nc.sync.dma_start(w[:], wq_r[:, t, :])
                                wq_t.append(w)
                            if True:
                                def qp_evac(h, m):
                                    # alternate ACT/DVE so evacuation keeps
                                    # pace with bank reuse
                                    if m % 2 == 0:
                                        nc.scalar.activation(
                                            qp[:, m, h * QB:(h + 1) * QB],
                                            pss[m][:], AF.Identity,
                                            bias=bq_t[:, m:m + 1], scale=ISCALE)
                                    else:
                                        nc.vector.tensor_scalar(
                                            qp[:, m, h * QB:(h + 1) * QB],
                                            pss[m][:], ISCALE,
                                            bq_t[:, m:m + 1],
                                            mybir.AluOpType.mult,
                                            mybir.AluOpType.add)

                                for h in range(NQB):
                                    pss = [ps8.tile([P, QB], f32,
                                                    tag=f"t{m % 4}",
                                                    name=f"qpps{h}_{m}")
                                           for m in range(ET)]
                                    for t in range(DT):
                                        for m in range(ET):
                                            mm(pss[m][:],
                                               wq_t[t][:, m * P:(m + 1) * P],
                                               qt_t[t][:, h * QB:(h + 1) * QB],
                                               t == 0, t == DT - 1)
                                            if t == DT - 1:
                                                qp_evac(h, m)

                        # vT loads on sync (after wq in queue order)
                        for mp in range(KT // 2):
                            nc.sync.dma_start(
                                vt_tiles[mp][:],
                                vT_r[:, :, mp * 2 * P:(mp + 1) * 2 * P])

                        if True:

                            # ---- kp phase: m-outer t-inner ----
                            if True:
                                for nb in range(NKB):
                                    kt_ = kt_tiles[nb]
                                    for m in range(ET):
                                        ps = ps8.tile([P, KNB], f32,
                                                      tag=f"t{m % 4}",
                                                      name=f"kpps{nb}_{m}")
                                        for t in range(DT):
                                            mm(ps[:],
                                               wk_t[t][:, m * P:(m + 1) * P],
                                               kt_[:, t, :], t == 0, t == DT - 1)
                                        nc.scalar.activation(
                                            kp[:, m, nb * KNB:(nb + 1) * KNB],
                                            ps[:], AF.Identity,
                                            bias=bk_t[:, m:m + 1])
                                    if nb + 2 < NKB:
                                        nc.gpsimd.dma_start(
                                            kt_tiles[nb + 2][:],
                                            kT_r[:, :, (nb + 2) * KNB:(nb + 3) * KNB])

                            # ---- vp phase ----
                            if True:
                                for mp in range(KT // 2):
                                    vt_ = vt_tiles[mp]
                                    for mh in range(2):
                                        m = mp * 2 + mh
                                        for n in range(E // ENB):
                                            ps = ps8.tile([P, ENB], f32,
                                                          tag=f"t{(2 * m + n) % 4}",
                                                          name=f"vpps{m}_{n}")
                                            for t in range(DT):
                                                mm(ps[:],
                                                   vt_[:, t, mh * P:(mh + 1) * P],
                                                   wv_t[t][:, n * ENB:(n + 1) * ENB],
                                                   t == 0, t == DT - 1)
                                            nc.vector.tensor_add(
                                                vp[:, m, n * ENB:(n + 1) * ENB],
                                                ps[:], bv_t[:, n * ENB:(n + 1) * ENB])

                    # streams + kv-era pad die here; attention gets its own pad
                    s1c_cm.__exit__(None, None, None)
                    s1b_cm.__exit__(None, None, None)
                    s1a_cm.__exit__(None, None, None)

                    if True:
                        if True:
                            # ---- attention + out, per q block ----
                            with tc.tile_pool(name="owp", bufs=1) as owp, \
                                 tc.tile_pool(name="osball", bufs=6) as osb:
                                ow_t = owp.tile([P, ET, D], bf16, name="ow_t")
                                for e in range(ET):
                                    nc.gpsimd.dma_start(ow_t[:, e, :],
                                                        ow_r[:, e, :])

                                for qb in range(NQB):
                                    q0 = qb * QB
                                    with tc.tile_pool(name=f"ctxs{qb}",
                                                      bufs=1) as ctxsp:
                                        ctx_sb = ctxsp.tile([P, ET, QB], bf16,
                                                            name=f"ctx{qb}")
                                        with tc.tile_pool(name=f"exp{qb}",
                                                          bufs=1) as expp:
                                            expT = expp.tile([P, KT, QB], bf16,
                                                             name=f"exp{qb}")
                                            exp8 = expp.tile(
                                                [P, KT, QB], mybir.dt.float8e4,
                                                name=f"exp8_{qb}")
                                            # logits + exp + lagged sum; the
                                            # last sum matmul moves into the
                                            # ctx loop below
                                            s_ps = ps8.tile([P, QB], f32,
                                                            tag="t1",
                                                            name=f"sps{qb}")
                                            def s_mm(pair, stop):
                                                # fp8 DoubleRow: 256-key
                                                # contraction per pass at
                                                # 0.5 cycles/row (errors on
                                                # the 2048-term positive sum
                                                # average down to ~0.1%)
                                                nc.tensor.matmul(
                                                    s_ps[:], ones8_t[:],
                                                    exp8[:, 2 * pair:2 * pair + 2, :],
                                                    start=pair == 0, stop=stop,
                                                    perf_mode=mybir.MatmulPerfMode.DoubleRow)

                                            for kb in range(KT):
                                                ps = ps8.tile([P, QB], f32,
                                                              tag="t0",
                                                              name=f"lg{qb}_{kb}")
                                                for e in range(ET):
                                                    mm(ps[:],
                                                       kp[:, e, kb * P:(kb + 1) * P],
                                                       qp[:, e, q0:q0 + QB],
                                                       e == 0, e == ET - 1)
                                                nc.scalar.activation(
                                                    expT[:, kb, :], ps[:], AF.Exp,
                                                    bias=mask_t[:, kb:kb + 1])
                                                nc.vector.tensor_scalar_mul(
                                                    exp8[:, kb, :],
                                                    expT[:, kb, :], 1.0)
                                                if kb >= 3 and kb % 2 == 1:
                                                    s_mm(kb // 2 - 1, False)
                                            # ctx accumulation (4 banks,
                                            # e%4); trailing sum matmul and
                                            # recip hide inside e=0
                                            for e in range(ET):
                                                cps = ps8.tile([P, QB], f32,
                                                               tag=f"t{2 + e % 2}",
                                                               name=f"ctxps{qb}_{e}")
                                                for kb in range(KT):
                                                    mm(cps[:],
                                                       vp[:, kb, e * P:(e + 1) * P],
                                                       expT[:, kb, :],
                                                       kb == 0, kb == KT - 1)
                                                    if e == 0 and kb == 6:
                                                        s_mm(KT // 2 - 1, True)
                                                if e == 0:
                                                    nc.vector.reciprocal(
                                                        recip_ts[qb][:], s_ps[:])
                                                nc.vector.tensor_mul(
                                                    ctx_sb[:, e, :], cps[:],
                                                    recip_ts[qb][:])
                                        # out projection: mq outer, nd inner
                                        if True:
                                            for mq in range(MQ):
                                                # last q-row block: 256-wide
                                                # chunks shorten the final
                                                # add+DMA tail
                                                fine = (qb == NQB - 1
                                                        and mq == MQ - 1)
                                                dnb = 256 if fine else DNB
                                                for nd in range(D // dnb):
                                                    ps = ps8.tile(
                                                        [P, dnb], f32,
                                                        tag="t0",
                                                        name=f"ops{qb}_{mq}_{nd}")
                                                    for e in range(ET):
                                                        mm(ps[:],
                                                           ctx_sb[:, e, mq * P:(mq + 1) * P],
                                                           ow_t[:, e, nd * dnb:(nd + 1) * dnb],
                                                           e == 0, e == ET - 1)
                                                    ot = osb.tile(
                                                        [P, dnb], f32, tag="ot",
                                                        name=f"ot{qb}_{mq}_{nd}")
                                                    nc.vector.tensor_add(
                                                        ot[:], ps[:],
                                                        ob_t[:, nd * dnb:(nd + 1) * dnb])
                                                    oeng = (nc.sync
                                                            if (mq + nd) % 2 == 0
                                                            else nc.scalar)
                                                    oeng.dma_start(
                                                        out[q0 + mq * P:q0 + (mq + 1) * P,
                                                            nd * dnb:(nd + 1) * dnb],
                                                        ot[:])

            ps8_cm.__exit__(None, None, None)

    nc.compile()
    return nc


def make_in_maps(v, k, q, mask, wq_w, wq_b, wk_w, wk_b, wv_w, wv_b, out_w, out_b,
                 n_cores=8, D=1024, E=1024, SK=2048, QSH=1024):
    """Host-side shard + layout prep (pure data movement / dtype cast)."""
    import ml_dtypes
    bf = ml_dtypes.bfloat16
    ET = E // P
    KT = SK // P
    f = np.float32
    wq_w = np.ascontiguousarray(np.asarray(wq_w, f).astype(bf))
    wk_w = np.ascontiguousarray(np.asarray(wk_w, f).astype(bf))
    wv_w = np.ascontiguousarray(np.asarray(wv_w, f).astype(bf))
    out_w = np.ascontiguousarray(np.asarray(out_w, f).astype(bf))
    iscale = np.float32(1.0 / np.sqrt(E))
    bq_col = np.ascontiguousarray(np.asarray(wq_b, f).reshape(ET, P).T * iscale)
    bk_col = np.ascontiguousarray(np.asarray(wk_b, f).reshape(ET, P).T)
    bv_bc = np.ascontiguousarray(np.broadcast_to(np.asarray(wv_b, f), (P, E)))
    ob_bc = np.ascontiguousarray(
        np.broadcast_to(np.asarray(out_b, f), (P, len(out_b))))
    ones_arr = np.ones((P, 256), bf)
    ones8_arr = np.ones((P, 256), ml_dtypes.float8_e4m3)
    in_maps = []
    for c in range(n_cores):
        b, h = divmod(c, 2)
        qTc = np.ascontiguousarray(
            np.asarray(q[b, h * QSH:(h + 1) * QSH, :], f).T.astype(bf))
        kTc = np.ascontiguousarray(np.asarray(k[b], f).T.astype(bf))
        vTc = np.ascontiguousarray(np.asarray(v[b], f).T.astype(bf))
        mc = np.ascontiguousarray(
            np.asarray(mask[b, 0], f).reshape(KT, P).T * np.float32(NEG))
        in_maps.append(dict(qT=qTc, kT=kTc, vT=vTc, mask_cols=mc,
                            ones_d=ones_arr, ones8_d=ones8_arr,
                            wq=wq_w, wk=wk_w, wv=wv_w, ow=out_w,
                            bq_col=bq_col, bk_col=bk_col,
                            bv_bc=bv_bc, ob_bc=ob_bc))
    return in_maps


_NC_CACHE = {}


def kernel(v, k, q, mask, wq_w, wq_b, wk_w, wk_b, wv_w, wv_b, out_w, out_b):
    from concourse.bass_utils import run_bass_kernel_spmd

    B, S, D = 4, 2048, 1024
    E, QSH = 1024, 1024
    if "nc" not in _NC_CACHE:
        _NC_CACHE["nc"] = build_nc(D=D, E=E, SK=S, QSH=QSH, QB=512)
    nc = _NC_CACHE["nc"]

    in_maps = make_in_maps(v, k, q, mask, wq_w, wq_b, wk_w, wk_b, wv_w, wv_b,
                           out_w, out_b, n_cores=8, D=D, E=E, SK=S, QSH=QSH)
    trace = bool(int(os.environ.get("BASS_KERNEL_TRACE", "0")))
    res = run_bass_kernel_spmd(nc, in_maps, core_ids=list(range(8)), trace=trace)
    if trace:
        print(f"HW exec time: {res.exec_time_ns} ns")
        _NC_CACHE["last_exec_time_ns"] = res.exec_time_ns
        _NC_CACHE["last_trace"] = res.instructions_and_trace

    outp = np.empty((B, S, D), np.float32)
    for c in range(8):
        b, h = divmod(c, 2)
        outp[b, h * QSH:(h + 1) * QSH, :] = res.results[c]["out"]
    return outp


# revision 38
# speedup vs baseline: 1.0013x; 1.0013x over previous
"""Single-head attention (B=4, S=2048, D=E=1024) on 8 trn2 NeuronCores.

Sharding: data-parallel over (batch, q-half) -> 8 shards. Each core gets a
1024-row q shard plus the full 2048 keys of its batch; K/V projections are
recomputed on both cores of a batch pair (zero collectives).

v5: PE-continuity design. One long gap-free matmul stream; DMA always ahead.
  - bf16 inputs (host-converted): halves DMA bytes + SBUF. Matmul rate is
    identical (1 cycle/row, keyed on moving-operand dtype).
  - single qp phase for BOTH q-halves (wq/qT loaded once)
  - ONE PSUM pool for the whole kernel (4 tags x ring-2 = 8 banks): per-slot
    ring deps instead of pool-close barriers (a close stalls the next
    phase's first matmul on the closed pool's LAST consumer)
  - DMA queues: wq alone on sync (SP owns the shared HWDGE's early window);
    qt/kT/wk/wv/ow on the gpsimd SWDGE in need order; smalls + half the out
    writes on scalar. The DMA pipe is serial and FIFO-by-gen-completion, so
    gen order == need order.
  - const-AP warm-up matmuls (zero deps) ramp the PE p-state during the
    ~4us DMA fill; count sized to end just before wq0 lands
  - softmax sums via fp8e4 DoubleRow (256-key contraction, 0.5 cyc/row):
    the 2048-term positive sum averages fp8 element error to ~0.1%. exp is
    written bf16 for ctx; DVE makes the fp8 shadow. Sum matmuls lag exp by
    two k-tiles; the final pair hides inside the ctx accumulation.
  - per q-block: logits -> exp -> ctx -> out fused; no DRAM ctx bounce;
    out staging in one persistent pool (close would barrier on DMA sems)

Per-core math (token-transposed on host so contraction lands on partitions):
  qp^T [E,q]   = (lhsT=wq[D,E], rhs=qT[D,q]) * (1/sqrt E) + bq/sqrt(E)
  kp^T [E,k]   = (lhsT=wk, rhs=kT) + bk
  vp   [k,E]   = (lhsT=vT[D,k], rhs=wv[D,E]) + bv
  lgT  [k,q]   = (lhsT=kp^T slice, rhs=qp^T)          (scale folded into qp)
  expT [k,q]   = Exp(lgT + mask*NEG)                  (ACT, per-partition bias)
  s    [.,q]   = ones-matmul over expT                (no max-sub: logits~N(0,1))
  ctx^T[E,q]   = (lhsT=vp slice, rhs=expT) * recip(s)
  out  [q,D]   = (lhsT=ctx^T slice, rhs=ow[E,D]) + ob
"""

import os
import numpy as np

P = 128
NEG = -1.0e9


def build_nc(D=1024, E=1024, SK=2048, QSH=1024, QB=512):
    """Build the per-core Bass module (SPMD; same program on all cores)."""
    import concourse.bass as bass
    import concourse.mybir as mybir
    import concourse.tile as tile
    from concourse import bacc

    f32 = mybir.dt.float32
    bf16 = mybir.dt.bfloat16
    AF = mybir.ActivationFunctionType

    DT = D // P          # contraction tiles over model dim
    ET = E // P          # enc tiles
    KT = SK // P         # key tiles
    NQB = QSH // QB      # q blocks
    KNB = 512            # key free-dim block for kp
    NKB = SK // KNB
    ENB = 512            # E free-dim block for vp
    DNB = 512            # model free-dim block for out
    MQ = QB // P
    ISCALE = 1.0 / float(np.sqrt(E))

    nc = bacc.Bacc(trn_type="TRN2")

    # ---- I/O ----
    qT = nc.dram_tensor("qT", [D, QSH], bf16, kind="ExternalInput")[:, :]
    kTd = nc.dram_tensor("kT", [D, SK], bf16, kind="ExternalInput")[:, :]
    vTd = nc.dram_tensor("vT", [D, SK], bf16, kind="ExternalInput")[:, :]
    mask_cols = nc.dram_tensor("mask_cols", [P, KT], f32, kind="ExternalInput")[:, :]
    ones_d = nc.dram_tensor("ones_d", [P, 256], bf16, kind="ExternalInput")[:, :]
    ones8_d = nc.dram_tensor("ones8_d", [P, 256], mybir.dt.float8e4,
                             kind="ExternalInput")[:, :]
    wq = nc.dram_tensor("wq", [D, E], bf16, kind="ExternalInput")[:, :]
    wk = nc.dram_tensor("wk", [D, E], bf16, kind="ExternalInput")[:, :]
    wv = nc.dram_tensor("wv", [D, E], bf16, kind="ExternalInput")[:, :]
    ow = nc.dram_tensor("ow", [E, D], bf16, kind="ExternalInput")[:, :]
    bq_col = nc.dram_tensor("bq_col", [P, ET], f32, kind="ExternalInput")[:, :]
    bk_col = nc.dram_tensor("bk_col", [P, ET], f32, kind="ExternalInput")[:, :]
    bv_bc = nc.dram_tensor("bv_bc", [P, E], f32, kind="ExternalInput")[:, :]
    ob_bc = nc.dram_tensor("ob_bc", [P, D], f32, kind="ExternalInput")[:, :]
    out = nc.dram_tensor("out", [QSH, D], f32, kind="ExternalOutput")[:, :]

    qT_r = qT.rearrange("(t p) n -> p t n", p=P)   # [128, DT, QSH]
    kT_r = kTd.rearrange("(t p) n -> p t n", p=P)
    vT_r = vTd.rearrange("(t p) n -> p t n", p=P)
    wq_r = wq.rearrange("(t p) n -> p t n", p=P)   # [128, DT, E]
    wk_r = wk.rearrange("(t p) n -> p t n", p=P)
    wv_r = wv.rearrange("(t p) n -> p t n", p=P)
    ow_r = ow.rearrange("(t p) n -> p t n", p=P)   # [128, ET, D]

    def mm(ps, lhsT, rhs, start, stop):
        nc.tensor.matmul(ps, lhsT, rhs, start=start, stop=stop)

    with tile.TileContext(nc) as tc:
        with tc.tile_pool(name="smalls", bufs=1) as smalls:
            # small persistent tensors on the scalar (ACT) DMA queue,
            # all emitted after the qt h0 halves (load_smalls_rest); mask/bq
            # arrive pre-multiplied by NEG / 1/sqrt(E) from the host.
            bq_t = smalls.tile([P, ET], f32, name="bqc")
            ones_t = smalls.tile([P, 256], bf16, name="ones")
            ones8_t = smalls.tile([P, 2, P], mybir.dt.float8e4, name="ones8")
            mask_t = smalls.tile([P, KT], f32, name="maskc")
            bk_t = smalls.tile([P, ET], f32, name="bkc")
            bv_t = smalls.tile([P, E], f32, name="bv_t")
            ob_t = smalls.tile([P, D], f32, name="ob_t")
            recip_ts = [smalls.tile([P, QB], f32, name=f"recip{i}")
                        for i in range(NQB)]

            def load_smalls_rest():
                # emitted after the qt h0 halves so they don't delay them
                nc.scalar.dma_start(bq_t[:], bq_col)
                nc.scalar.dma_start(ones_t[:], ones_d)
                nc.scalar.dma_start(ones8_t[:], ones8_d)
                nc.scalar.dma_start(mask_t[:], mask_cols)
                nc.scalar.dma_start(bk_t[:], bk_col)
                nc.scalar.dma_start(bv_t[:], bv_bc)
                nc.scalar.dma_start(ob_t[:], ob_bc)

            # warm-up matmuls read the preamble-materialized const-1.0 AP
            # (broadcast [128,1] -> [128,256]): zero runtime dependencies, so
            # the PE ramps its p-state from t~0 while the DMA pipe fills.
            wu_src = nc.const_aps.tensor(1.0, (P, 256), bf16)

            # ONE psum pool for the whole kernel: 4 tags x ring-2 = 8 banks.
            # Per-slot ring deps replace pool-close barriers (a pool close
            # stalls the next phase's first matmul on the closed pool's LAST
            # consumer; the ring only waits on the 2-back tile's consumers).
            ps8_cm = tc.tile_pool(name="ps8", bufs=2, space="PSUM")
            ps8 = ps8_cm.__enter__()
            for i in range(12):
                wt = ps8.tile([P, 256], f32, tag=f"t{i % 4}", name=f"wu{i}")
                mm(wt[:], wu_src[:, :P], wu_src, True, True)

            with tc.tile_pool(name="qppool", bufs=1) as qp_pool:
                qp = qp_pool.tile([P, ET, QSH], bf16, name="qp")
                with tc.tile_pool(name="kvp", bufs=1) as kvp:
                    kp = kvp.tile([P, ET, SK], bf16, name="kp")
                    vp = kvp.tile([P, KT, E], bf16, name="vp")

                    s1a_cm = tc.tile_pool(name="s1a", bufs=1)
                    s1b_cm = tc.tile_pool(name="s1b", bufs=2)
                    s1c_cm = tc.tile_pool(name="s1c", bufs=3)
                    s1a = s1a_cm.__enter__()
                    s1b = s1b_cm.__enter__()
                    s1c = s1c_cm.__enter__()
                    if True:
                        # gpsimd queue order (need order): qt h0, qt h1,
                        # kT b0, wk, kT b1, wv; kT b2/b3 are emitted inside
                        # the kp loop so their ring wait can't block wv/ow.
                        qt_t = []
                        for t in range(DT):
                            qt = s1a.tile([P, QSH], bf16, name=f"qt{t}")
                            nc.gpsimd.dma_start(qt[:, :QB], qT_r[:, t, :QB])
                            qt_t.append(qt)
                        for t in range(DT):
                            nc.gpsimd.dma_start(qt_t[t][:, QB:],
                                                qT_r[:, t, QB:])
                        kt_tiles = [s1b.tile([P, DT, KNB], bf16, tag="ktb",
                                             name=f"kt{nb}")
                                    for nb in range(NKB)]
                        # kT b0 before wk (single gen, needed at the same
                        # time); both after the qt halves so their transfers
                        # miss the qp phase's wq window
                        nc.gpsimd.dma_start(kt_tiles[0][:], kT_r[:, :, :KNB])
                        wk_t = []
                        for t in range(DT):
                            w = s1a.tile([P, E], bf16, name=f"wk{t}")
                            nc.gpsimd.dma_start(w[:], wk_r[:, t, :])
                            wk_t.append(w)
                        nc.gpsimd.dma_start(kt_tiles[1][:],
                                            kT_r[:, :, KNB:2 * KNB])
                        wv_t = []
                        for t in range(DT):
                            w = s1a.tile([P, E], bf16, name=f"wv{t}")
                            nc.gpsimd.dma_start(w[:], wv_r[:, t, :])
                            wv_t.append(w)
                        # vT ring tiles; DMAs go on the sync queue after wq
                        vt_tiles = []
                        for mp in range(KT // 2):
                            vt_ = s1c.tile([P, DT, 2 * P], bf16, tag="vt",
                                           name=f"vt{mp}")
                            vt_tiles.append(vt_)

                        # ---- qp phase: both q halves; wq on sync queue ----
                        with tc.tile_pool(name="s2", bufs=1) as s2:
                            wq_t = []
                            for t in range(DT):
                                w = s2.tile([P, E], bf16, name=f"wq{t}")
                                if t == 0:
                                    # halves on parallel queues: m0-3 gate on
                                    # 128KB via sync while scalar's first gen
                                    # delivers m4-7's half concurrently
                                    nc.sync.dma_start(w[:, :E // 2],
                                                      wq_r[:, t, :E // 2])
                                    nc.scalar.dma_start(w[:, E // 2:],
                                                        wq_r[:, t, E // 2:])
                                else:
                                    nc.sync.dma_start(w[:], wq_r[:, t, :])
                                wq_t.append(w)
                            load_smalls_rest()
                            if True:
                                def qp_evac(h, m):
                                    # alternate ACT/DVE so evacuation keeps
                                    # pace with bank reuse
                                    if m % 2 == 0:
                                        nc.scalar.activation(
                                            qp[:, m, h * QB:(h + 1) * QB],
                                            pss[m][:], AF.Identity,
                                            bias=bq_t[:, m:m + 1], scale=ISCALE)
                                    else:
                                        nc.vector.tensor_scalar(
                                            qp[:, m, h * QB:(h + 1) * QB],
                                            pss[m][:], ISCALE,
                                            bq_t[:, m:m + 1],
                                            mybir.AluOpType.mult,
                                            mybir.AluOpType.add)

                                for h in range(NQB):
                                    pss = [ps8.tile([P, QB], f32,
                                                    tag=f"t{m % 4}",
                                                    name=f"qpps{h}_{m}")
                                           for m in range(ET)]
                                    for t in range(DT):
                                        for m in range(ET):
                                            mm(pss[m][:],
                                               wq_t[t][:, m * P:(m + 1) * P],
                                               qt_t[t][:, h * QB:(h + 1) * QB],
                                               t == 0, t == DT - 1)
                                            if t == DT - 1:
                                                qp_evac(h, m)

                        # vT loads on sync (after wq in queue order)
                        for mp in range(KT // 2):
                            nc.sync.dma_start(
                                vt_tiles[mp][:],
                                vT_r[:, :, mp * 2 * P:(mp + 1) * 2 * P])

                        if True:

                            # ---- kp phase: m-outer t-inner ----
                            if True:
                                for nb in range(NKB):
                                    kt_ = kt_tiles[nb]
                                    for m in range(ET):
                                        ps = ps8.tile([P, KNB], f32,
                                                      tag=f"t{m % 4}",
                                                      name=f"kpps{nb}_{m}")
                                        for t in range(DT):
                                            mm(ps[:],
                                               wk_t[t][:, m * P:(m + 1) * P],
                                               kt_[:, t, :], t == 0, t == DT - 1)
                                        nc.scalar.activation(
                                            kp[:, m, nb * KNB:(nb + 1) * KNB],
                                            ps[:], AF.Identity,
                                            bias=bk_t[:, m:m + 1])
                                    if nb + 2 < NKB:
                                        nc.gpsimd.dma_start(
                                            kt_tiles[nb + 2][:],
                                            kT_r[:, :, (nb + 2) * KNB:(nb + 3) * KNB])

                            # ---- vp phase ----
                            if True:
                                for mp in range(KT // 2):
                                    vt_ = vt_tiles[mp]
                                    for mh in range(2):
                                        m = mp * 2 + mh
                                        for n in range(E // ENB):
                                            ps = ps8.tile([P, ENB], f32,
                                                          tag=f"t{(2 * m + n) % 4}",
                                                          name=f"vpps{m}_{n}")
                                            for t in range(DT):
                                                mm(ps[:],
                                                   vt_[:, t, mh * P:(mh + 1) * P],
                                                   wv_t[t][:, n * ENB:(n + 1) * ENB],
                                                   t == 0, t == DT - 1)
                                            nc.vector.tensor_add(
                                                vp[:, m, n * ENB:(n + 1) * ENB],
                                                ps[:], bv_t[:, n * ENB:(n + 1) * ENB])

                    # streams + kv-era pad die here; attention gets its own pad
                    s1c_cm.__exit__(None, None, None)
                    s1b_cm.__exit__(None, None, None)
                    s1a_cm.__exit__(None, None, None)

                    if True:
                        if True:
                            # ---- attention + out, per q block ----
                            with tc.tile_pool(name="owp", bufs=1) as owp, \
                                 tc.tile_pool(name="osball", bufs=6) as osb:
                                ow_t = owp.tile([P, ET, D], bf16, name="ow_t")
                                for e in range(ET):
                                    nc.gpsimd.dma_start(ow_t[:, e, :],
                                                        ow_r[:, e, :])

                                for qb in range(NQB):
                                    q0 = qb * QB
                                    with tc.tile_pool(name=f"ctxs{qb}",
                                                      bufs=1) as ctxsp:
                                        ctx_sb = ctxsp.tile([P, ET, QB], bf16,
                                                            name=f"ctx{qb}")
                                        with tc.tile_pool(name=f"exp{qb}",
                                                          bufs=1) as expp:
                                            expT = expp.tile([P, KT, QB], bf16,
                                                             name=f"exp{qb}")
                                            exp8 = expp.tile(
                                                [P, KT, QB], mybir.dt.float8e4,
                                                name=f"exp8_{qb}")
                                            # logits + exp + lagged sum; the
                                            # last sum matmul moves into the
                                            # ctx loop below
                                            s_ps = ps8.tile([P, QB], f32,
                                                            tag="t1",
                                                            name=f"sps{qb}")
                                            def s_mm(pair, stop):
                                                # fp8 DoubleRow: 256-key
                                                # contraction per pass at
                                                # 0.5 cycles/row (errors on
                                                # the 2048-term positive sum
                                                # average down to ~0.1%)
                                                nc.tensor.matmul(
                                                    s_ps[:], ones8_t[:],
                                                    exp8[:, 2 * pair:2 * pair + 2, :],
                                                    start=pair == 0, stop=stop,
                                                    perf_mode=mybir.MatmulPerfMode.DoubleRow)

                                            for kb in range(KT):
                                                ps = ps8.tile([P, QB], f32,
                                                              tag="t0",
                                                              name=f"lg{qb}_{kb}")
                                                for e in range(ET):
                                                    mm(ps[:],
                                                       kp[:, e, kb * P:(kb + 1) * P],
                                                       qp[:, e, q0:q0 + QB],
                                                       e == 0, e == ET - 1)
                                                nc.scalar.activation(
                                                    expT[:, kb, :], ps[:], AF.Exp,
                                                    bias=mask_t[:, kb:kb + 1])
                                                nc.vector.tensor_scalar_mul(
                                                    exp8[:, kb, :],
                                                    expT[:, kb, :], 1.0)
                                                if kb >= 3 and kb % 2 == 1:
                                                    s_mm(kb // 2 - 1, False)
                                            # ctx accumulation (4 banks,
                                            # e%4); trailing sum matmul and
                                            # recip hide inside e=0
                                            for e in range(ET):
                                                cps = ps8.tile([P, QB], f32,
                                                               tag=f"t{2 + e % 2}",
                                                               name=f"ctxps{qb}_{e}")
                                                for kb in range(KT):
                                                    mm(cps[:],
                                                       vp[:, kb, e * P:(e + 1) * P],
                                                       expT[:, kb, :],
                                                       kb == 0, kb == KT - 1)
                                                    if e == 0 and kb == 6:
                                                        s_mm(KT // 2 - 1, True)
                                                if e == 0:
                                                    nc.vector.reciprocal(
                                                        recip_ts[qb][:], s_ps[:])
                                                nc.vector.tensor_mul(
                                                    ctx_sb[:, e, :], cps[:],
                                                    recip_ts[qb][:])
                                        # out projection: mq outer, nd inner
                                        if True:
                                            for mq in range(MQ):
                                                # last q-row block: 256-wide
                                                # chunks shorten the final
                                                # add+DMA tail
                                                fine = (qb == NQB - 1
                                                        and mq == MQ - 1)
                                                dnb = 256 if fine else DNB
                                                for nd in range(D // dnb):
                                                    ps = ps8.tile(
                                                        [P, dnb], f32,
                                                        tag="t0",
                                                        name=f"ops{qb}_{mq}_{nd}")
                                                    for e in range(ET):
                                                        mm(ps[:],
                                                           ctx_sb[:, e, mq * P:(mq + 1) * P],
                                                           ow_t[:, e, nd * dnb:(nd + 1) * dnb],
                                                           e == 0, e == ET - 1)
                                                    ot = osb.tile(
                                                        [P, dnb], f32, tag="ot",
                                                        name=f"ot{qb}_{mq}_{nd}")
                                                    nc.vector.tensor_add(
                                                        ot[:], ps[:],
                                                        ob_t[:, nd * dnb:(nd + 1) * dnb])
                                                    oeng = (nc.sync
                                                            if (mq + nd) % 2 == 0
                                                            else nc.scalar)
                                                    oeng.dma_start(
                                                        out[q0 + mq * P:q0 + (mq + 1) * P,
                                                            nd * dnb:(nd + 1) * dnb],
                                                        ot[:])

            ps8_cm.__exit__(None, None, None)

    nc.compile()
    return nc


def make_in_maps(v, k, q, mask, wq_w, wq_b, wk_w, wk_b, wv_w, wv_b, out_w, out_b,
                 n_cores=8, D=1024, E=1024, SK=2048, QSH=1024):
    """Host-side shard + layout prep (pure data movement / dtype cast)."""
    import ml_dtypes
    bf = ml_dtypes.bfloat16
    ET = E // P
    KT = SK // P
    f = np.float32
    wq_w = np.ascontiguousarray(np.asarray(wq_w, f).astype(bf))
    wk_w = np.ascontiguousarray(np.asarray(wk_w, f).astype(bf))
    wv_w = np.ascontiguousarray(np.asarray(wv_w, f).astype(bf))
    out_w = np.ascontiguousarray(np.asarray(out_w, f).astype(bf))
    iscale = np.float32(1.0 / np.sqrt(E))
    bq_col = np.ascontiguousarray(np.asarray(wq_b, f).reshape(ET, P).T * iscale)
    bk_col = np.ascontiguousarray(np.asarray(wk_b, f).reshape(ET, P).T)
    bv_bc = np.ascontiguousarray(np.broadcast_to(np.asarray(wv_b, f), (P, E)))
    ob_bc = np.ascontiguousarray(
        np.broadcast_to(np.asarray(out_b, f), (P, len(out_b))))
    ones_arr = np.ones((P, 256), bf)
    ones8_arr = np.ones((P, 256), ml_dtypes.float8_e4m3)
    in_maps = []
    for c in range(n_cores):
        b, h = divmod(c, 2)
        qTc = np.ascontiguousarray(
            np.asarray(q[b, h * QSH:(h + 1) * QSH, :], f).T.astype(bf))
        kTc = np.ascontiguousarray(np.asarray(k[b], f).T.astype(bf))
        vTc = np.ascontiguousarray(np.asarray(v[b], f).T.astype(bf))
        mc = np.ascontiguousarray(
            np.asarray(mask[b, 0], f).reshape(KT, P).T * np.float32(NEG))
        in_maps.append(dict(qT=qTc, kT=kTc, vT=vTc, mask_cols=mc,
                            ones_d=ones_arr, ones8_d=ones8_arr,
                            wq=wq_w, wk=wk_w, wv=wv_w, ow=out_w,
                            bq_col=bq_col, bk_col=bk_col,
                            bv_bc=bv_bc, ob_bc=ob_bc))
    return in_maps


_NC_CACHE = {}


def kernel(v, k, q, mask, wq_w, wq_b, wk_w, wk_b, wv_w, wv_b, out_w, out_b):
    from concourse.bass_utils import run_bass_kernel_spmd

    B, S, D = 4, 2048, 1024
    E, QSH = 1024, 1024
    if "nc" not in _NC_CACHE:
        _NC_CACHE["nc"] = build_nc(D=D, E=E, SK=S, QSH=QSH, QB=512)
    nc = _NC_CACHE["nc"]

    in_maps = make_in_maps(v, k, q, mask, wq_w, wq_b, wk_w, wk_b, wv_w, wv_b,
                           out_w, out_b, n_cores=8, D=D, E=E, SK=S, QSH=QSH)
    trace = bool(int(os.environ.get("BASS_KERNEL_TRACE", "0")))
    res = run_bass_kernel_spmd(nc, in_maps, core_ids=list(range(8)), trace=trace)
    if trace:
        print(f"HW exec time: {res.exec_time_ns} ns")
        _NC_CACHE["last_exec_time_ns"] = res.exec_time_ns
        _NC_CACHE["last_trace"] = res.instructions_and_trace

    outp = np.empty((B, S, D), np.float32)
    for c in range(8):
        b, h = divmod(c, 2)
        outp[b, h * QSH:(h + 1) * QSH, :] = res.results[c]["out"]
    return outp
